# revision 19
# baseline (speedup 1.0000x reference)
"""Trainium2 Bass kernel for nn_MHA_2688649527670.

Reference computes, per batch b and head h:
    Q = x Wq_h^T, K = x Wk_h^T, V = x Wv_h^T          ([S, D] each)
    Z = softmax_over_d( (Q K^T / sqrt(D)) V )

No softmax between Q K^T and V, so the chain is associative:
    (Q K^T) V = x (Wq_h^T Wk_h G Wv_h^T) / sqrt(D),   G = x^T x   ([D, D])

which collapses the O(S^2 D) attention into a [D,D] weight chain plus one
[S,D]x[D,D*H] matmul, then softmax over d (free axis). Per-head softmax bias
is mandatory: per-head logit scales differ by >1000x, so a shared row max
underflows weak heads.

Sharding: batch (4) x head-groups (2x4 heads) = 8 independent cores.

Perf notes:
  - finals/UT in float32r: ~1 cycle/row at N=512 vs 4 for fp32, and HW
    measures ~1.5e-4 matmul error (~16x better than bf16; bf16 finals fail
    the 2e-2 gate at 3.1e-2, f32r lands ~2e-3). f32r operands must be
    WRITTEN as f32r by their producer (BIR rule); psum->sbuf copies do it.
  - PE p-state warmup matmuls on a memset tile from t~0 (streak -> 2.4GHz).
  - PE order: G (DMA-paced) with xT transposes interleaved, then P0T/WvT
    (weights arrive late on the gpsimd queue - off critical path), UT, M.
  - epilogue per chunk: V reduce_max -> 4x scalar Exp (per-head bias) ->
    V reduce_sum -> paired V reciprocal -> gpsimd normalize-mult (bf16) ->
    s-major contiguous bf16 DMA out (host reorders/upcasts).
"""

import ml_dtypes
import numpy as np

import concourse.bass as bass
import concourse.bacc as bacc
import concourse.mybir as mybir
import concourse.tile as tile
from concourse.bass_utils import run_bass_kernel_spmd
from concourse.masks import make_identity

B, S, D, H = 4, 2048, 128, 8
P = 128
HPC = H // 2          # heads per core
NCH = S // P          # 16 s-chunks of 128 rows
N_CORES = 8
SCALE = 1.0 / float(np.sqrt(D))
F32 = mybir.dt.float32
F32R = mybir.dt.float32r
BF16 = mybir.dt.bfloat16

FINALS_F32R = True
UT_F32R = True
N_WARM = 4

_PROG = None


def _build_program():
    nc = bacc.Bacc("TRN2", target_bir_lowering=False, debug=False,
                   num_devices=N_CORES)

    x_d = nc.dram_tensor("x", [S, D], F32, kind="ExternalInput")
    wq_d = nc.dram_tensor("wq", [HPC * D, D], F32, kind="ExternalInput")
    wk_d = nc.dram_tensor("wk", [HPC * D, D], F32, kind="ExternalInput")
    wv_d = nc.dram_tensor("wv", [HPC * D, D], F32, kind="ExternalInput")
    emask_d = nc.dram_tensor("emask", [HPC, HPC * D], BF16,
                             kind="ExternalInput")
    # s-major output: [chunk, s-in-chunk, head, d], bf16; host reorders
    out_d = nc.dram_tensor("out", [NCH, P, HPC, D], BF16, kind="ExternalOutput")

    XT_DT = F32R if FINALS_F32R else F32

    with tile.TileContext(nc) as tc:
        with (
            tc.tile_pool(name="const", bufs=1) as const,
            tc.tile_pool(name="work", bufs=6) as work,
            tc.tile_pool(name="small", bufs=4) as small,
            tc.tile_pool(name="ps_y", bufs=3, space="PSUM") as ps_y,
            tc.tile_pool(name="ps_g", bufs=1, space="PSUM") as ps_g,
            tc.tile_pool(name="ps_t", bufs=2, space="PSUM") as ps_t,
            tc.tile_pool(name="ps_mx", bufs=2, space="PSUM") as ps_mx,
        ):
            ident = const.tile([P, P], F32, tag="ident")
            make_identity(nc, ident)

            # ---- input DMAs: x on the two HW queues, weights on gpsimd ----
            x_sb = const.tile([P, NCH, D], F32, tag="x_sb")
            x_view = x_d.ap().rearrange("(n p) c -> p n c", p=P)
            for q in range(8):
                eng = nc.sync if q % 2 == 0 else nc.scalar
                eng.dma_start(x_sb[:, 2 * q:2 * q + 2, :],
                              x_view[:, 2 * q:2 * q + 2, :])
            w_sb = {}
            for nm, wd in (("wq", wq_d), ("wk", wk_d), ("wv", wv_d)):
                t = const.tile([P, HPC, D], F32, tag=f"{nm}_sb", name=f"{nm}_sb")
                nc.gpsimd.dma_start(t, wd.ap().rearrange("(h p) c -> p h c", p=P))
                w_sb[nm] = t

            # ---- G = x^T x (DMA-paced) with xT transposes interleaved.
            #      Transposes land 4-per-PSUM-bank; one V cast per bank
            #      writes the f32r xT tile (16 copies -> 4). ----
            g_ps = ps_g.tile([P, P], F32, tag="g_ps")
            xT_sb = const.tile([P, NCH, D], XT_DT, tag="xT_sb")
            tp_banks = {}

            def emit_xt(i):
                b = i // 4
                if i % 4 == 0:
                    tp_banks[b] = ps_t.tile([P, 4, P], F32, tag="tp", name=f"tp{b}")
                nc.tensor.transpose(tp_banks[b][:, i % 4, :],
                                    x_sb[:, i, :], ident)

            def emit_cast(b):
                if b % 2 == 0:
                    nc.vector.tensor_copy(xT_sb[:, 4 * b:4 * b + 4, :],
                                          tp_banks[b])
                else:
                    nc.scalar.copy(xT_sb[:, 4 * b:4 * b + 4, :], tp_banks[b])

            p0t_ps = ps_y.tile([P, HPC * D], F32, tag="c_ps")
            for i in range(NCH):
                nc.tensor.matmul(g_ps, lhsT=x_sb[:, i, :], rhs=x_sb[:, i, :],
                                 start=(i == 0), stop=(i == NCH - 1))
                if i >= 4:
                    emit_xt(i - 4)
                if i >= 12:  # P0T woven into the G tail (weights ready)
                    h = i - 12
                    nc.tensor.matmul(p0t_ps[:, h * D:(h + 1) * D],
                                     lhsT=w_sb["wk"][:, h, :],
                                     rhs=w_sb["wq"][:, h, :])
            # chain-critical copies FIRST so the G/P0T semaphores are
            #      consumed promptly; xT tail and WvT fill PE gaps after
            g_sb = const.tile([P, P], F32R if UT_F32R else F32, tag="g_sb")
            nc.vector.tensor_copy(g_sb, g_ps)
            p0t_sb = const.tile([P, HPC * D], F32R if UT_F32R else F32,
                                tag="p0t_sb")
            nc.vector.tensor_copy(p0t_sb, p0t_ps)

            for i in range(NCH - 4, NCH):
                emit_xt(i)
            wvt_ps = ps_y.tile([P, HPC * D], F32, tag="c_ps")
            for h in range(HPC):
                nc.tensor.transpose(wvt_ps[:, h * D:(h + 1) * D],
                                    w_sb["wv"][:, h, :], ident)
            wvt_sb = const.tile([P, HPC * D], F32, tag="wvt_sb")
            nc.scalar.copy(wvt_sb, wvt_ps)

            # ---- UT = G @ P0T (G symmetric), one N=512 f32r matmul ----
            ut_ps = ps_y.tile([P, HPC * D], F32, tag="c_ps")
            nc.tensor.matmul(ut_ps, lhsT=g_sb, rhs=p0t_sb)
            ut_sb = const.tile([P, HPC * D], F32, tag="ut_sb")
            nc.vector.tensor_copy(ut_sb, ut_ps)

            # ---- M_h = UT_h^T WvT_h, scaled on the scalar engine ----
            m_ps = ps_y.tile([P, HPC * D], F32, tag="c_ps")
            for h in range(HPC):
                sl = slice(h * D, (h + 1) * D)
                nc.tensor.matmul(m_ps[:, sl], lhsT=ut_sb[:, sl],
                                 rhs=wvt_sb[:, sl])
            m_all = const.tile([P, HPC * D], XT_DT, tag="m_all")
            nc.scalar.mul(m_all, m_ps, SCALE)

            # xT psum->sbuf casts (V even banks, S odd) after chain copies
            for b in range(4):
                emit_cast(b)

            # ---- finals + 3-stage pipelined epilogue. Per-head max is
            #      subtracted INSIDE PSUM by a tiny K=4 bf16 accumulate-
            #      matmul (lhsT = bf16(-max)^T, rhs = head block mask), so
            #      ONE zero-bias Exp per chunk replaces four per-head ones;
            #      the bf16 max rounding cancels exactly in the ratio. ----
            emask = const.tile([HPC, HPC * D], BF16, tag="emask")
            nc.gpsimd.dma_start(emask, emask_d.ap())

            y_live, nm_live, t_live = {}, {}, {}

            def emit_front(i):
                y_ps = ps_y.tile([P, HPC * D], F32, tag="c_ps")
                nc.tensor.matmul(y_ps, lhsT=xT_sb[:, i, :], rhs=m_all[:])
                negmax = small.tile([P, HPC], F32, tag="negmax")
                nc.vector.reduce_max(
                    out=negmax,
                    in_=y_ps[:].rearrange("p (h d) -> p h d", h=HPC),
                    axis=mybir.AxisListType.X, negate=True)
                y_live[i], nm_live[i] = y_ps, negmax

            def emit_mid(i):
                y_ps, negmax = y_live.pop(i), nm_live.pop(i)
                mx_ps = ps_mx.tile([HPC, P], F32, tag="mx")
                nc.tensor.transpose(mx_ps, negmax, ident)
                mx_bf = small.tile([HPC, P], BF16, tag="mx_bf")
                nc.scalar.copy(mx_bf, mx_ps)
                nc.tensor.matmul(y_ps, lhsT=mx_bf, rhs=emask[:],
                                 start=False, stop=True)
                t_sb = work.tile([P, HPC, D], F32, tag="t_sb")
                nc.scalar.activation(
                    t_sb[:].rearrange("p h d -> p (h d)"), y_ps[:],
                    mybir.ActivationFunctionType.Exp, bias=0.0, scale=1.0)
                t_live[i] = t_sb

            def emit_back(i):
                t_sb = t_live.pop(i)
                sums = small.tile([P, HPC], F32, tag="sums")
                nc.vector.reduce_sum(out=sums, in_=t_sb,
                                     axis=mybir.AxisListType.X)
                rsum = small.tile([P, HPC], F32, tag="rsum")
                nc.vector.reciprocal(rsum, sums)
                o_sb = work.tile([P, HPC, D], BF16, tag="o_sb")
                nc.gpsimd.tensor_tensor(
                    o_sb, t_sb,
                    rsum[:, :, None].to_broadcast((P, HPC, D)),
                    mybir.AluOpType.mult)
                eng = nc.sync if i % 2 == 0 else nc.scalar
                eng.dma_start(out_d.ap()[i], o_sb)

            emit_front(0)
            for i in range(1, NCH):
                emit_front(i)
                emit_mid(i - 1)
                if i >= 2:
                    emit_back(i - 2)
            emit_mid(NCH - 1)
            emit_back(NCH - 2)
            emit_back(NCH - 1)

    nc.compile()
    return nc


def _get_program():
    global _PROG
    if _PROG is None:
        _PROG = _build_program()
    return _PROG


def _make_in_maps(x, W_q, W_k, W_v):
    in_maps = []
    for core in range(N_CORES):
        b, hg = core // 2, core % 2
        sl = slice(hg * HPC * D, (hg + 1) * HPC * D)
        em = np.zeros((HPC, HPC * D), np.float32)
        for j in range(HPC):
            em[j, j * D:(j + 1) * D] = 1.0
        in_maps.append({
            "x": np.ascontiguousarray(x[b]),
            "emask": em.astype(ml_dtypes.bfloat16),
            "wq": np.ascontiguousarray(W_q[sl]),
            "wk": np.ascontiguousarray(W_k[sl]),
            "wv": np.ascontiguousarray(W_v[sl]),
        })
    return in_maps


def run(x, W_q, W_k, W_v, trace=False, **spmd_kwargs):
    """Run on 8 NeuronCores; returns (Z, BassKernelResults)."""
    nc = _get_program()
    in_maps = _make_in_maps(np.asarray(x, np.float32), np.asarray(W_q, np.float32),
                            np.asarray(W_k, np.float32), np.asarray(W_v, np.float32))
    res = run_bass_kernel_spmd(nc, in_maps, core_ids=list(range(N_CORES)),
                               trace=trace, **spmd_kwargs)
    Z = np.empty((B, H, S, D), np.float32)
    for core in range(N_CORES):
        b, hg = core // 2, core % 2
        o = np.asarray(res.results[core]["out"]).astype(np.float32)
        # [NCH, P, HPC, D] -> [HPC, NCH*P, D]
        Z[b, hg * HPC:(hg + 1) * HPC] = o.transpose(2, 0, 1, 3).reshape(HPC, S, D)
    return Z, res


def kernel(x, W_q, W_k, W_v):
    Z, _ = run(x, W_q, W_k, W_v, trace=False)
    return Z


# revision 21
# speedup vs baseline: 1.0565x; 1.0565x over previous
"""Trainium2 Bass kernel for nn_MHA_2688649527670.

Reference computes, per batch b and head h:
    Q = x Wq_h^T, K = x Wk_h^T, V = x Wv_h^T          ([S, D] each)
    Z = softmax_over_d( (Q K^T / sqrt(D)) V )

No softmax between Q K^T and V, so the chain is associative:
    (Q K^T) V = x (Wq_h^T Wk_h G Wv_h^T) / sqrt(D),   G = x^T x   ([D, D])

which collapses the O(S^2 D) attention into a [D,D] weight chain plus one
[S,D]x[D,D*H] matmul, then softmax over d (free axis). Per-head softmax bias
is mandatory: per-head logit scales differ by >1000x, so a shared row max
underflows weak heads.

Sharding: batch (4) x head-groups (2x4 heads) = 8 independent cores.

Perf notes:
  - finals/UT in float32r: ~1 cycle/row at N=512 vs 4 for fp32, and HW
    measures ~1.5e-4 matmul error (~16x better than bf16; bf16 finals fail
    the 2e-2 gate at 3.1e-2, f32r lands ~2e-3). f32r operands must be
    WRITTEN as f32r by their producer (BIR rule); psum->sbuf copies do it.
  - PE p-state warmup matmuls on a memset tile from t~0 (streak -> 2.4GHz).
  - PE order: G (DMA-paced) with xT transposes interleaved, then P0T/WvT
    (weights arrive late on the gpsimd queue - off critical path), UT, M.
  - epilogue per chunk: V reduce_max -> 4x scalar Exp (per-head bias) ->
    V reduce_sum -> paired V reciprocal -> gpsimd normalize-mult (bf16) ->
    s-major contiguous bf16 DMA out (host reorders/upcasts).
"""

import ml_dtypes
import numpy as np

import concourse.bass as bass
import concourse.bacc as bacc
import concourse.mybir as mybir
import concourse.tile as tile
from concourse.bass_utils import run_bass_kernel_spmd
from concourse.masks import make_identity

B, S, D, H = 4, 2048, 128, 8
P = 128
HPC = H // 2          # heads per core
NCH = S // P          # 16 s-chunks of 128 rows
N_CORES = 8
SCALE = 1.0 / float(np.sqrt(D))
F32 = mybir.dt.float32
F32R = mybir.dt.float32r
BF16 = mybir.dt.bfloat16

FINALS_F32R = True
UT_F32R = True
N_WARM = 4

_PROG = None


def _build_program():
    nc = bacc.Bacc("TRN2", target_bir_lowering=False, debug=False,
                   num_devices=N_CORES)

    x_d = nc.dram_tensor("x", [S, D], F32, kind="ExternalInput")
    wq_d = nc.dram_tensor("wq", [HPC * D, D], F32, kind="ExternalInput")
    wk_d = nc.dram_tensor("wk", [HPC * D, D], F32, kind="ExternalInput")
    wv_d = nc.dram_tensor("wv", [HPC * D, D], F32, kind="ExternalInput")
    # s-major output: [chunk, s-in-chunk, head, d], bf16; host reorders
    out_d = nc.dram_tensor("out", [NCH, P, HPC, D], BF16, kind="ExternalOutput")

    XT_DT = F32R if FINALS_F32R else F32

    with tile.TileContext(nc) as tc:
        with (
            tc.tile_pool(name="const", bufs=1) as const,
            tc.tile_pool(name="work", bufs=6) as work,
            tc.tile_pool(name="small", bufs=4) as small,
            tc.tile_pool(name="ps_y", bufs=4, space="PSUM") as ps_y,
            tc.tile_pool(name="ps_g", bufs=1, space="PSUM") as ps_g,
            tc.tile_pool(name="ps_t", bufs=2, space="PSUM") as ps_t,
        ):
            ident = const.tile([P, P], F32, tag="ident")
            make_identity(nc, ident)

            # ---- input DMAs: x on the two HW queues, weights on gpsimd ----
            x_sb = const.tile([P, NCH, D], F32, tag="x_sb")
            x_view = x_d.ap().rearrange("(n p) c -> p n c", p=P)
            engs = [nc.sync, nc.scalar, nc.sync, nc.scalar, nc.sync,
                    nc.scalar, nc.gpsimd, nc.gpsimd]
            for q in range(8):
                engs[q].dma_start(x_sb[:, 2 * q:2 * q + 2, :],
                                  x_view[:, 2 * q:2 * q + 2, :])
            w_sb = {}
            for nm, wd in (("wq", wq_d), ("wk", wk_d), ("wv", wv_d)):
                t = const.tile([P, HPC, D], F32, tag=f"{nm}_sb", name=f"{nm}_sb")
                nc.gpsimd.dma_start(t, wd.ap().rearrange("(h p) c -> p h c", p=P))
                w_sb[nm] = t

            # ---- G = x^T x (DMA-paced) with xT transposes interleaved.
            #      Transposes land 4-per-PSUM-bank; one V cast per bank
            #      writes the f32r xT tile (16 copies -> 4). ----
            g_ps = ps_g.tile([P, P], F32, tag="g_ps")
            xT_sb = const.tile([P, NCH, D], XT_DT, tag="xT_sb")
            tp_banks = {}

            def emit_xt(i):
                b = i // 4
                if i % 4 == 0:
                    tp_banks[b] = ps_t.tile([P, 4, P], F32, tag="tp", name=f"tp{b}")
                nc.tensor.transpose(tp_banks[b][:, i % 4, :],
                                    x_sb[:, i, :], ident)

            def emit_cast(b):
                if b % 2 == 0:
                    nc.vector.tensor_copy(xT_sb[:, 4 * b:4 * b + 4, :],
                                          tp_banks[b])
                else:
                    nc.scalar.copy(xT_sb[:, 4 * b:4 * b + 4, :], tp_banks[b])

            p0t_ps = ps_y.tile([P, HPC * D], F32, tag="c_ps")
            for i in range(NCH):
                nc.tensor.matmul(g_ps, lhsT=x_sb[:, i, :], rhs=x_sb[:, i, :],
                                 start=(i == 0), stop=(i == NCH - 1))
                if i >= 4:
                    emit_xt(i - 4)
                if i >= 12:  # P0T woven into the G tail (weights ready)
                    h = i - 12
                    nc.tensor.matmul(p0t_ps[:, h * D:(h + 1) * D],
                                     lhsT=w_sb["wk"][:, h, :],
                                     rhs=w_sb["wq"][:, h, :])
            # chain-critical copies FIRST so the G/P0T semaphores are
            #      consumed promptly; xT tail and WvT fill PE gaps after
            g_sb = const.tile([P, P], F32R if UT_F32R else F32, tag="g_sb")
            nc.vector.tensor_copy(g_sb, g_ps)
            p0t_sb = const.tile([P, HPC * D], F32R if UT_F32R else F32,
                                tag="p0t_sb")
            nc.vector.tensor_copy(p0t_sb, p0t_ps)

            for i in range(NCH - 4, NCH):
                emit_xt(i)
            wvt_ps = ps_y.tile([P, HPC * D], F32, tag="c_ps")
            for h in range(HPC):
                nc.tensor.transpose(wvt_ps[:, h * D:(h + 1) * D],
                                    w_sb["wv"][:, h, :], ident)
            wvt_sb = const.tile([P, HPC * D], F32, tag="wvt_sb")
            nc.scalar.copy(wvt_sb, wvt_ps)

            # ---- UT = G @ P0T (G symmetric), one N=512 f32r matmul ----
            ut_ps = ps_y.tile([P, HPC * D], F32, tag="c_ps")
            nc.tensor.matmul(ut_ps, lhsT=g_sb, rhs=p0t_sb)
            ut_sb = const.tile([P, HPC * D], F32, tag="ut_sb")
            nc.vector.tensor_copy(ut_sb, ut_ps)

            # ---- M_h = UT_h^T WvT_h, scaled on the scalar engine ----
            m_ps = ps_y.tile([P, HPC * D], F32, tag="c_ps")
            for h in range(HPC):
                sl = slice(h * D, (h + 1) * D)
                nc.tensor.matmul(m_ps[:, sl], lhsT=ut_sb[:, sl],
                                 rhs=wvt_sb[:, sl])
            m_all = const.tile([P, HPC * D], XT_DT, tag="m_all")
            nc.scalar.mul(m_all, m_ps, SCALE)

            # xT psum->sbuf casts (V even banks, S odd) after chain copies
            for b in range(4):
                emit_cast(b)

            # ---- finals + software-pipelined softmax epilogue:
            #      V runs max_{i} before sum_{i-1} so it never idles on S ----
            t_live = {}

            def emit_front(i):
                y_ps = ps_y.tile([P, HPC * D], F32, tag="c_ps")
                nc.tensor.matmul(y_ps, lhsT=xT_sb[:, i, :], rhs=m_all[:])
                negmax = small.tile([P, HPC], F32, tag="negmax")
                nc.vector.reduce_max(
                    out=negmax,
                    in_=y_ps[:].rearrange("p (h d) -> p h d", h=HPC),
                    axis=mybir.AxisListType.X, negate=True)
                t_sb = work.tile([P, HPC, D], F32, tag="t_sb")
                for h in range(HPC):
                    nc.scalar.activation(
                        t_sb[:, h, :], y_ps[:, h * D:(h + 1) * D],
                        mybir.ActivationFunctionType.Exp,
                        bias=negmax[:, h:h + 1], scale=1.0)
                t_live[i] = t_sb

            sums_live, pend = {}, {}

            def emit_sum(i):
                if i % 2 == 0:
                    sums_live[i // 2] = small.tile([P, 2, HPC], F32,
                                                   tag="sums", name="sums")
                nc.vector.reduce_sum(out=sums_live[i // 2][:, i % 2],
                                     in_=t_live[i], axis=mybir.AxisListType.X)
                if i % 2 == 1:
                    rsum = small.tile([P, 2, HPC], F32, tag="rsum")
                    nc.vector.reciprocal(rsum, sums_live.pop(i // 2))
                    pend[i - 1] = pend[i] = rsum

            def emit_out(i):
                t_sb, rsum = t_live.pop(i), pend.pop(i)
                o_sb = work.tile([P, HPC, D], BF16, tag="o_sb")
                nc.gpsimd.tensor_tensor(
                    o_sb, t_sb,
                    rsum[:, i % 2][:, :, None].to_broadcast((P, HPC, D)),
                    mybir.AluOpType.mult)
                eng = nc.sync if i % 2 == 0 else nc.scalar
                eng.dma_start(out_d.ap()[i], o_sb)

            emit_front(0)
            emit_front(1)
            emit_sum(0)
            for i in range(2, NCH):
                emit_front(i)
                emit_sum(i - 1)
                if (i - 1) % 2 == 1:
                    emit_out(i - 2)
                    emit_out(i - 1)
            emit_sum(NCH - 1)
            emit_out(NCH - 2)
            emit_out(NCH - 1)

    nc.compile()
    return nc


def _get_program():
    global _PROG
    if _PROG is None:
        _PROG = _build_program()
    return _PROG


def _make_in_maps(x, W_q, W_k, W_v):
    in_maps = []
    for core in range(N_CORES):
        b, hg = core // 2, core % 2
        sl = slice(hg * HPC * D, (hg + 1) * HPC * D)
        in_maps.append({
            "x": np.ascontiguousarray(x[b]),
            "wq": np.ascontiguousarray(W_q[sl]),
            "wk": np.ascontiguousarray(W_k[sl]),
            "wv": np.ascontiguousarray(W_v[sl]),
        })
    return in_maps


def run(x, W_q, W_k, W_v, trace=False, **spmd_kwargs):
    """Run on 8 NeuronCores; returns (Z, BassKernelResults)."""
    nc = _get_program()
    in_maps = _make_in_maps(np.asarray(x, np.float32), np.asarray(W_q, np.float32),
                            np.asarray(W_k, np.float32), np.asarray(W_v, np.float32))
    res = run_bass_kernel_spmd(nc, in_maps, core_ids=list(range(N_CORES)),
                               trace=trace, **spmd_kwargs)
    Z = np.empty((B, H, S, D), np.float32)
    for core in range(N_CORES):
        b, hg = core // 2, core % 2
        o = np.asarray(res.results[core]["out"]).astype(np.float32)
        # [NCH, P, HPC, D] -> [HPC, NCH*P, D]
        Z[b, hg * HPC:(hg + 1) * HPC] = o.transpose(2, 0, 1, 3).reshape(HPC, S, D)
    return Z, res


def kernel(x, W_q, W_k, W_v):
    Z, _ = run(x, W_q, W_k, W_v, trace=False)
    return Z


# revision 23
# speedup vs baseline: 1.0903x; 1.0320x over previous
"""Trainium2 Bass kernel for nn_MHA_2688649527670.

Reference computes, per batch b and head h:
    Q = x Wq_h^T, K = x Wk_h^T, V = x Wv_h^T          ([S, D] each)
    Z = softmax_over_d( (Q K^T / sqrt(D)) V )

No softmax between Q K^T and V, so the chain is associative:
    (Q K^T) V = x (Wq_h^T Wk_h G Wv_h^T) / sqrt(D),   G = x^T x   ([D, D])

which collapses the O(S^2 D) attention into a [D,D] weight chain plus one
[S,D]x[D,D*H] matmul, then softmax over d (free axis). Per-head softmax bias
is mandatory: per-head logit scales differ by >1000x, so a shared row max
underflows weak heads.

Sharding: batch (4) x head-groups (2x4 heads) = 8 independent cores.

Perf notes:
  - finals/UT in float32r: ~1 cycle/row at N=512 vs 4 for fp32, and HW
    measures ~1.5e-4 matmul error (~16x better than bf16; bf16 finals fail
    the 2e-2 gate at 3.1e-2, f32r lands ~2e-3). f32r operands must be
    WRITTEN as f32r by their producer (BIR rule); psum->sbuf copies do it.
  - PE p-state warmup matmuls on a memset tile from t~0 (streak -> 2.4GHz).
  - PE order: G (DMA-paced) with xT transposes interleaved, then P0T/WvT
    (weights arrive late on the gpsimd queue - off critical path), UT, M.
  - epilogue per chunk: V reduce_max -> 4x scalar Exp (per-head bias) ->
    V reduce_sum -> paired V reciprocal -> gpsimd normalize-mult (bf16) ->
    s-major contiguous bf16 DMA out (host reorders/upcasts).
"""

import ml_dtypes
import numpy as np

import concourse.bass as bass
import concourse.bacc as bacc
import concourse.mybir as mybir
import concourse.tile as tile
from concourse.bass_utils import run_bass_kernel_spmd
from concourse.masks import make_identity

B, S, D, H = 4, 2048, 128, 8
P = 128
HPC = H // 2          # heads per core
NCH = S // P          # 16 s-chunks of 128 rows
N_CORES = 8
SCALE = 1.0 / float(np.sqrt(D))
F32 = mybir.dt.float32
F32R = mybir.dt.float32r
BF16 = mybir.dt.bfloat16

FINALS_F32R = True
UT_F32R = True
N_WARM = 4

_PROG = None


def _build_program():
    nc = bacc.Bacc("TRN2", target_bir_lowering=False, debug=False,
                   num_devices=N_CORES)

    x_d = nc.dram_tensor("x", [S, D], F32, kind="ExternalInput")
    wq_d = nc.dram_tensor("wq", [HPC * D, D], F32, kind="ExternalInput")
    wk_d = nc.dram_tensor("wk", [HPC * D, D], F32, kind="ExternalInput")
    wv_d = nc.dram_tensor("wv", [HPC * D, D], F32, kind="ExternalInput")
    # s-major output: [chunk, s-in-chunk, head, d], bf16; host reorders
    out_d = nc.dram_tensor("out", [NCH, P, HPC, D], BF16, kind="ExternalOutput")

    XT_DT = F32R if FINALS_F32R else F32

    with tile.TileContext(nc) as tc:
        with (
            tc.tile_pool(name="const", bufs=1) as const,
            tc.tile_pool(name="work", bufs=6) as work,
            tc.tile_pool(name="small", bufs=4) as small,
            tc.tile_pool(name="ps_y", bufs=4, space="PSUM") as ps_y,
            tc.tile_pool(name="ps_g", bufs=1, space="PSUM") as ps_g,
            tc.tile_pool(name="ps_t", bufs=2, space="PSUM") as ps_t,
        ):
            ident = const.tile([P, P], F32, tag="ident")
            make_identity(nc, ident)

            # ---- input DMAs: x on the two HW queues, weights on gpsimd ----
            x_sb = const.tile([P, NCH, D], F32, tag="x_sb")
            x_view = x_d.ap().rearrange("(n p) c -> p n c", p=P)
            for q in range(8):
                eng = nc.sync if q % 2 == 0 else nc.scalar
                eng.dma_start(x_sb[:, 2 * q:2 * q + 2, :],
                              x_view[:, 2 * q:2 * q + 2, :])
            w_sb = {}
            for nm, wd in (("wq", wq_d), ("wk", wk_d), ("wv", wv_d)):
                t = const.tile([P, HPC, D], F32, tag=f"{nm}_sb", name=f"{nm}_sb")
                nc.gpsimd.dma_start(t, wd.ap().rearrange("(h p) c -> p h c", p=P))
                w_sb[nm] = t

            # ---- G = x^T x (DMA-paced) with xT transposes interleaved.
            #      Transposes land 4-per-PSUM-bank; one V cast per bank
            #      writes the f32r xT tile (16 copies -> 4). ----
            g_ps = ps_g.tile([P, P], F32, tag="g_ps")
            xT_sb = const.tile([P, NCH, D], XT_DT, tag="xT_sb")
            tp_banks = {}

            def emit_xt(i):
                b = i // 4
                if i % 4 == 0:
                    tp_banks[b] = ps_t.tile([P, 4, P], F32, tag="tp", name=f"tp{b}")
                nc.tensor.transpose(tp_banks[b][:, i % 4, :],
                                    x_sb[:, i, :], ident)

            def emit_cast(b):
                if b % 2 == 0:
                    nc.vector.tensor_copy(xT_sb[:, 4 * b:4 * b + 4, :],
                                          tp_banks[b])
                else:
                    nc.scalar.copy(xT_sb[:, 4 * b:4 * b + 4, :], tp_banks[b])

            p0t_ps = ps_y.tile([P, HPC * D], F32, tag="c_ps")
            for i in range(NCH):
                nc.tensor.matmul(g_ps, lhsT=x_sb[:, i, :], rhs=x_sb[:, i, :],
                                 start=(i == 0), stop=(i == NCH - 1))
                if i >= 4:
                    emit_xt(i - 4)
                if i >= 12:  # P0T woven into the G tail (weights ready)
                    h = i - 12
                    nc.tensor.matmul(p0t_ps[:, h * D:(h + 1) * D],
                                     lhsT=w_sb["wk"][:, h, :],
                                     rhs=w_sb["wq"][:, h, :])
            # chain-critical copies FIRST so the G/P0T semaphores are
            #      consumed promptly; xT tail and WvT fill PE gaps after
            g_sb = const.tile([P, P], F32R if UT_F32R else F32, tag="g_sb")
            nc.vector.tensor_copy(g_sb, g_ps)
            p0t_sb = const.tile([P, HPC * D], F32R if UT_F32R else F32,
                                tag="p0t_sb")
            nc.vector.tensor_copy(p0t_sb, p0t_ps)
            # banks 0/1 feed finals 0-7: cast them before the chain tail
            emit_cast(0)
            emit_cast(1)

            for i in range(NCH - 4, NCH):
                emit_xt(i)
            wvt_ps = ps_y.tile([P, HPC * D], F32, tag="c_ps")
            for h in range(HPC):
                nc.tensor.transpose(wvt_ps[:, h * D:(h + 1) * D],
                                    w_sb["wv"][:, h, :], ident)
            wvt_sb = const.tile([P, HPC * D], F32, tag="wvt_sb")
            nc.scalar.copy(wvt_sb, wvt_ps)

            # ---- UT = G @ P0T (G symmetric), one N=512 f32r matmul ----
            ut_ps = ps_y.tile([P, HPC * D], F32, tag="c_ps")
            nc.tensor.matmul(ut_ps, lhsT=g_sb, rhs=p0t_sb)
            ut_sb = const.tile([P, HPC * D], F32, tag="ut_sb")
            nc.vector.tensor_copy(ut_sb, ut_ps)

            # ---- M_h = UT_h^T WvT_h, scaled on the scalar engine ----
            m_ps = ps_y.tile([P, HPC * D], F32, tag="c_ps")
            for h in range(HPC):
                sl = slice(h * D, (h + 1) * D)
                nc.tensor.matmul(m_ps[:, sl], lhsT=ut_sb[:, sl],
                                 rhs=wvt_sb[:, sl])
            m_all = const.tile([P, HPC * D], XT_DT, tag="m_all")
            nc.scalar.mul(m_all, m_ps, SCALE)

            # late banks feed finals 8-15 only
            emit_cast(2)
            emit_cast(3)

            # ---- finals + software-pipelined softmax epilogue:
            #      V runs max_{i} before sum_{i-1} so it never idles on S ----
            t_live = {}

            def emit_front(i):
                y_ps = ps_y.tile([P, HPC * D], F32, tag="c_ps")
                nc.tensor.matmul(y_ps, lhsT=xT_sb[:, i, :], rhs=m_all[:])
                negmax = small.tile([P, HPC], F32, tag="negmax")
                nc.vector.reduce_max(
                    out=negmax,
                    in_=y_ps[:].rearrange("p (h d) -> p h d", h=HPC),
                    axis=mybir.AxisListType.X, negate=True)
                t_sb = work.tile([P, HPC, D], F32, tag="t_sb")
                for h in range(HPC):
                    nc.scalar.activation(
                        t_sb[:, h, :], y_ps[:, h * D:(h + 1) * D],
                        mybir.ActivationFunctionType.Exp,
                        bias=negmax[:, h:h + 1], scale=1.0)
                t_live[i] = t_sb

            def emit_back(i):
                t_sb = t_live.pop(i)
                sums = small.tile([P, HPC], F32, tag="sums")
                nc.vector.reduce_sum(out=sums, in_=t_sb,
                                     axis=mybir.AxisListType.X)
                rsum = small.tile([P, HPC], F32, tag="rsum")
                nc.vector.reciprocal(rsum, sums)
                o_sb = work.tile([P, HPC, D], BF16, tag="o_sb")
                nc.gpsimd.tensor_tensor(
                    o_sb, t_sb,
                    rsum[:, :, None].to_broadcast((P, HPC, D)),
                    mybir.AluOpType.mult)
                eng = nc.sync if i % 2 == 0 else nc.scalar
                eng.dma_start(out_d.ap()[i], o_sb)

            emit_front(0)
            for i in range(1, NCH):
                emit_front(i)
                emit_back(i - 1)
            emit_back(NCH - 1)

    nc.compile()
    return nc


def _get_program():
    global _PROG
    if _PROG is None:
        _PROG = _build_program()
    return _PROG


def _make_in_maps(x, W_q, W_k, W_v):
    in_maps = []
    for core in range(N_CORES):
        b, hg = core // 2, core % 2
        sl = slice(hg * HPC * D, (hg + 1) * HPC * D)
        in_maps.append({
            "x": np.ascontiguousarray(x[b]),
            "wq": np.ascontiguousarray(W_q[sl]),
            "wk": np.ascontiguousarray(W_k[sl]),
            "wv": np.ascontiguousarray(W_v[sl]),
        })
    return in_maps


def run(x, W_q, W_k, W_v, trace=False, **spmd_kwargs):
    """Run on 8 NeuronCores; returns (Z, BassKernelResults)."""
    nc = _get_program()
    in_maps = _make_in_maps(np.asarray(x, np.float32), np.asarray(W_q, np.float32),
                            np.asarray(W_k, np.float32), np.asarray(W_v, np.float32))
    res = run_bass_kernel_spmd(nc, in_maps, core_ids=list(range(N_CORES)),
                               trace=trace, **spmd_kwargs)
    Z = np.empty((B, H, S, D), np.float32)
    for core in range(N_CORES):
        b, hg = core // 2, core % 2
        o = np.asarray(res.results[core]["out"]).astype(np.float32)
        # [NCH, P, HPC, D] -> [HPC, NCH*P, D]
        Z[b, hg * HPC:(hg + 1) * HPC] = o.transpose(2, 0, 1, 3).reshape(HPC, S, D)
    return Z, res


def kernel(x, W_q, W_k, W_v):
    Z, _ = run(x, W_q, W_k, W_v, trace=False)
    return Z
